# revision 9
# baseline (speedup 1.0000x reference)
"""Trainium2 Bass kernel for nn_BatchConv1d (dynamic per-query conv kernels + banded conv).

Reference computation (per batch b):
    G[i, o]   = (q[b] @ Wk.T + bk)[i, o],  o = c*3 + t   (per-query dynamic kernels)
    bias[i]   = (q[b] @ Wb.T + bb)[i, 0]
    scores[i, j] = sum_{c,t} G[i, c*3+t] * k_pad[b, j+t, c]
    out = scores + bias[:, None] + bias_b

Associativity restructure + Winograd F(2,3) on the banded stage:
    stage 1:  N[s, j] = sum_{c,t} Wk[3c+t, s] * k_pad[j+t, c]
              via F(2,3): for output pairs (j=2m, 2m+1) define
                d0=kp[2m], d1=kp[2m+1], d2=kp[2m+2], d3=kp[2m+3]
                D0 = d0-d2, D1 = d1+d2, D2 = d2-d1, D3 = d1-d3   (DVE adds)
                M_u[s, m] = sum_c G_u[c, s] * D_u[m, c]          (4 GEMMs, PE)
                  with G0=g0, G1=(g0+g1+g2)/2, G2=(g0-g1+g2)/2, G3=g2 (host)
                Ne = M0+M1+M2 (even j), No = M1-M2-M3 (odd j)
                  drain: ACT evicts M1,M2 to bf16; DVE combines, with the
                  M0/M3 terms folded into scalar_tensor_tensor PSUM reads
              -> 64 MMs/batch instead of 96 (1.5x fewer stage-1 FLOPs)
    stage 2:  scores = q @ N  (64 MMs/batch); bias[i] rides the eviction
              (even halves on ACT, odd halves on DVE); rank-1 bk-term r[j]
              added on the host after the gather.
    Output is stored j-deinterleaved ([even block | odd block]); the host
    re-interleaves during the gather (free).

Input DMA is consumption-ordered on one ring (delivery is HBM-limited at
~370 GB/s): per c-chunk kb then G-lo (s-chunks 0-1), then the G-hi halves,
then qT/bias/batch-1 k. Batch-0 stage 1 runs (sh0,sh1) jointly contraction-
outermost (each arriving chunk feeds 8 MMs), sh2 contraction-outermost on the
arriving G-hi stream, sh3 u-outermost so its bank drains stagger into stage 2.

Sharding: batch data-parallel, 2 batches per core across 8 NeuronCores.
Compute dtype: bf16 matmul inputs, fp32 PSUM accumulation, bf16 output.
"""
import ml_dtypes
import numpy as np

from concourse import bacc, tile, mybir
from concourse.bass_utils import run_bass_kernel_spmd

BF16 = mybir.dt.bfloat16
F32 = mybir.dt.float32
Identity = mybir.ActivationFunctionType.Identity
ADD = mybir.AluOpType.add
MULT = mybir.AluOpType.mult

B, QL, KL, QS, KS, KW = 16, 1024, 1024, 512, 512, 3
NCORES = 8
B_LOC = B // NCORES      # 2 batches per core
NC_C = KS // 128         # 4 chunks of the c contraction dim
NC_S = QS // 128         # 4 s-chunks
NI = QL // 128           # 8 i-chunks
M = KL // 2              # 512 Winograd output pairs
KT = M + 1               # 513 ke/ko columns (incl. the pad sample)
KB = 2 * KT              # 1026 = ke|ko block per c-chunk
GH = 4 * 256             # 1024 cols per G half (4 u's x 256 s)
CWK = KB + 2 * GH        # 3074 packed kb+G width per c-chunk (batch 0)

_NC_CACHE = {}


def _build():
    nc = bacc.Bacc("TRN2", target_bir_lowering=False, debug=False)
    # host-prepped layouts (bf16 unless noted):
    #   kwg [p=c', c*CWK + 0:513]                    ke of batch 0: kp[2m, 128c+p]
    #       [p=c', c*CWK + 513 + m]                  ko of batch 0: kp[2m+1, .]
    #       [p=c', c*CWK + 1026 + u*256 + s']        G_u[128c+p, s], s in [0,256)
    #       [p=c', c*CWK + 2050 + u*256 + s']        G_u[128c+p, s], s in [256,512)
    #   kt1 [p=c', c*1026 + {ke|ko}]                 batch 1 ke/ko
    #   qT  [b][p=s', sc*1024 + i]                   q transposed, s-major
    #   bc  [p=i', b*8+ih]  f32                      bias col (+bb+bias_b)
    kwg_d = nc.declare_dram_parameter("kwg", [128, NC_C * CWK], BF16, isOutput=False)
    kt1_d = nc.declare_dram_parameter("kt1", [128, NC_C * KB], BF16, isOutput=False)
    qT_d = nc.declare_dram_parameter("qT", [B_LOC, 128, NC_S * QL], BF16, isOutput=False)
    bc_d = nc.declare_dram_parameter("bc", [128, B_LOC * NI], F32, isOutput=False)
    # out cols 0:512 = even j, 512:1024 = odd j (host re-interleaves)
    out_d = nc.declare_dram_parameter("out", [B_LOC, QL, KL], BF16, isOutput=True)

    with tile.TileContext(nc) as tc:
        with (
            tc.tile_pool(name="const", bufs=1) as cpool,
            tc.tile_pool(name="qin", bufs=2) as qpool,
            tc.tile_pool(name="din", bufs=2) as dpool,
            tc.tile_pool(name="cev", bufs=2) as cepool,
            tc.tile_pool(name="tmp", bufs=2) as tpool,
            tc.tile_pool(name="nst", bufs=2) as npool,
            tc.tile_pool(name="outp", bufs=3) as opool,
            tc.tile_pool(name="ps", bufs=2, space="PSUM") as ps,
        ):
            # ---- PE warmup: junk matmuls on a gpsimd-memset tile so the HAM
            #      clock gate flips while the first input chunks are in flight ----
            wz_sb = cpool.tile([128, 640], BF16)
            nc.gpsimd.memset(wz_sb[:], 0.0)
            wps = ps.tile([128, 512], F32, tag="u3", name="wps")
            for _ in range(7):
                nc.tensor.matmul(wps[:], wz_sb[:, 0:128], wz_sb[:, 128:640],
                                 start=True, stop=True)

            # ---- input DMAs: one ring (sync HWDGE), strict FIFO in
            #      consumption order: (kb, G-lo) per c-chunk, G-hi per
            #      c-chunk, then qT0 / bias / batch-1 kb / qT1 ----
            kwg = cpool.tile([128, NC_C * CWK], BF16)
            for c in range(NC_C):
                nc.sync.dma_start(kwg[:, c * CWK:c * CWK + KB],
                                  kwg_d[:, c * CWK:c * CWK + KB])
                nc.sync.dma_start(kwg[:, c * CWK + KB:c * CWK + KB + GH],
                                  kwg_d[:, c * CWK + KB:c * CWK + KB + GH])
            for c in range(NC_C):
                nc.sync.dma_start(kwg[:, c * CWK + KB + GH:(c + 1) * CWK],
                                  kwg_d[:, c * CWK + KB + GH:(c + 1) * CWK])
            qT = {}
            for b in range(B_LOC):
                qT[b] = qpool.tile([128, NC_S * QL], BF16, tag="qTb", name=f"qT{b}")
            nc.sync.dma_start(qT[0][:], qT_d[0, :, :])
            bc_sb = cpool.tile([128, B_LOC * NI], F32)
            nc.sync.dma_start(bc_sb[:], bc_d[:])
            kt1 = cpool.tile([128, NC_C * KB], BF16)
            nc.sync.dma_start(kt1[:], kt1_d[:])
            nc.sync.dma_start(qT[1][:], qT_d[1, :, :])

            def g_ap(u, c, sh):
                off = c * CWK + KB + (sh // 2) * GH + u * 256 + (sh % 2) * 128
                return kwg[:, off:off + 128]

            for b in range(B_LOC):
                def ke_ap(c, lo, hi):
                    base = (c * CWK) if b == 0 else (c * KB)
                    src = kwg if b == 0 else kt1
                    return src[:, base + lo:base + hi]

                def ko_ap(c, lo, hi):
                    base = ((c * CWK) if b == 0 else (c * KB)) + KT
                    src = kwg if b == 0 else kt1
                    return src[:, base + lo:base + hi]

                # ---- D-transform: D_u per c-chunk, 4 ops each, split
                #      DVE/GpSimd so the per-chunk latency halves ----
                D = dpool.tile([128, NC_C * 2048], BF16, tag="D", name=f"D{b}")

                def d_ap(c, u):
                    off = c * 2048 + u * 512
                    return D[:, off:off + 512]

                for c in range(NC_C):
                    nc.vector.tensor_sub(d_ap(c, 0), ke_ap(c, 0, 512), ke_ap(c, 1, 513))
                    nc.vector.tensor_add(d_ap(c, 1), ko_ap(c, 0, 512), ke_ap(c, 1, 513))
                    nc.vector.tensor_sub(d_ap(c, 2), ke_ap(c, 1, 513), ko_ap(c, 0, 512))
                    nc.vector.tensor_sub(d_ap(c, 3), ko_ap(c, 0, 512), ko_ap(c, 1, 513))

                # ---- stage 1: M_u[s,m] accumulated over c into 4 PSUM banks
                #      per s-chunk; drain: ACT evicts M1/M2 to bf16, DVE does
                #      p12=c1+c2, Ne=M0+p12, m12=c1-c2, No=m12-M3 (the M0/M3
                #      reads folded into scalar_tensor_tensor) ----
                Ne, No = {}, {}

                def drain(sh, banks):
                    c1 = cepool.tile([128, 512], BF16, tag="c1", name="c1")
                    c2 = cepool.tile([128, 512], BF16, tag="c2", name="c2")
                    nc.scalar.activation(c1[:], banks[1][:], Identity)
                    nc.scalar.activation(c2[:], banks[2][:], Identity)
                    p12 = tpool.tile([128, 512], BF16, tag="p12")
                    m12 = tpool.tile([128, 512], BF16, tag="m12")
                    ne = npool.tile([128, 512], BF16, tag=f"ne{sh}")
                    no = npool.tile([128, 512], BF16, tag=f"no{sh}")
                    nc.vector.tensor_add(p12[:], c1[:], c2[:])
                    nc.vector.scalar_tensor_tensor(ne[:], banks[0][:], 1.0, p12[:],
                                                   MULT, ADD)
                    nc.vector.tensor_sub(m12[:], c1[:], c2[:])
                    nc.vector.scalar_tensor_tensor(no[:], banks[3][:], -1.0, m12[:],
                                                   MULT, ADD)
                    Ne[sh], No[sh] = ne, no

                def new_banks(sh):
                    return [ps.tile([128, 512], F32, tag=f"u{u}", name=f"n{sh}{u}")
                            for u in range(4)]

                if b == 0:
                    # lead-in: sh0+sh1 jointly, contraction outermost so each
                    # arriving (kb, G-lo) c-chunk feeds 8 MMs immediately
                    nps = {sh: new_banks(sh) for sh in (0, 1)}
                    for c in range(NC_C):
                        for sh in (0, 1):
                            for u in range(4):
                                nc.tensor.matmul(nps[sh][u][:], g_ap(u, c, sh),
                                                 d_ap(c, u),
                                                 start=(c == 0), stop=(c == NC_C - 1))
                    drain(0, nps[0])
                    drain(1, nps[1])
                    # sh2 contraction-outermost, paced by the arriving G-hi
                    banks2 = new_banks(2)
                    for c in range(NC_C):
                        for u in range(4):
                            nc.tensor.matmul(banks2[u][:], g_ap(u, c, 2), d_ap(c, u),
                                             start=(c == 0), stop=(c == NC_C - 1))
                    drain(2, banks2)
                    rest = (3,)
                else:
                    rest = (0, 1, 2, 3)
                for sh in rest:
                    # u-outer so the 4 banks stop staggered and the drain
                    # overlaps this s-chunk's own MMs
                    banks = new_banks(sh)
                    for u in range(4):
                        for c in range(NC_C):
                            nc.tensor.matmul(banks[u][:], g_ap(u, c, sh), d_ap(c, u),
                                             start=(c == 0), stop=(c == NC_C - 1))
                    drain(sh, banks)

                # ---- stage 2: out = q @ N + bias[i]; even halves evicted on
                #      ACT, odd halves on DVE; host re-interleaves ----
                for i in range(NI):
                    last = b == B_LOC - 1 and i == NI - 1
                    osb = opool.tile([128, KL], BF16, tag="osb")
                    col = bc_sb[:, b * NI + i: b * NI + i + 1]
                    if not last:
                        for half in range(2):
                            Nl = Ne if half == 0 else No
                            bank = ps.tile([128, 512], F32,
                                           tag=f"u{2 * (i % 2) + half}")
                            for sc in range(NC_S):
                                nc.tensor.matmul(
                                    bank[:],
                                    qT[b][:, sc * QL + i * 128: sc * QL + (i + 1) * 128],
                                    Nl[sc][:],
                                    start=(sc == 0), stop=(sc == NC_S - 1),
                                )
                            if half == 0:
                                nc.scalar.activation(osb[:, 0:512], bank[:],
                                                     Identity, bias=col)
                            else:
                                nc.vector.tensor_scalar_add(osb[:, 512:1024],
                                                            bank[:], col)
                        nc.sync.dma_start(out_d[b, i * 128:(i + 1) * 128, :], osb[:])
                    else:
                        # final tile: four N=256 quarter-chains so the
                        # epilogue+store pipeline drains right behind the MMs;
                        # even quarters on ACT+sync ring, odd on DVE+scalar ring
                        for half in range(2):
                            Nl = Ne if half == 0 else No
                            for qh in range(2):
                                bank = ps.tile([128, 256], F32,
                                               tag=f"u{2 * half + qh}",
                                               name=f"fin{half}{qh}")
                                for sc in range(NC_S):
                                    nc.tensor.matmul(
                                        bank[:],
                                        qT[b][:, sc * QL + i * 128:
                                               sc * QL + (i + 1) * 128],
                                        Nl[sc][:, qh * 256:(qh + 1) * 256],
                                        start=(sc == 0), stop=(sc == NC_S - 1),
                                    )
                                lo = half * 512 + qh * 256
                                if half == 0:
                                    nc.scalar.activation(osb[:, lo:lo + 256],
                                                         bank[:], Identity, bias=col)
                                else:
                                    nc.vector.tensor_scalar_add(osb[:, lo:lo + 256],
                                                                bank[:], col)
                                ring = nc.sync if half == 0 else nc.scalar
                                ring.dma_start(
                                    out_d[b, i * 128:(i + 1) * 128, lo:lo + 256],
                                    osb[:, lo:lo + 256],
                                )
    nc.finalize()
    return nc


def _get_nc():
    if "nc" not in _NC_CACHE:
        _NC_CACHE["nc"] = _build()
    return _NC_CACHE["nc"]


def _prep_in_maps(q, k, Wk, bk, Wb, bb, bias_b):
    """Returns (in_maps, r) where r[B, KL] must be added to the reordered output."""
    bf16 = ml_dtypes.bfloat16
    q = np.asarray(q, dtype=np.float32)
    k = np.asarray(k, dtype=np.float32)
    Wk = np.asarray(Wk, dtype=np.float32)
    bk = np.asarray(bk, dtype=np.float32)
    Wb = np.asarray(Wb, dtype=np.float32)
    bb = np.asarray(bb, dtype=np.float32)
    bias_b = np.asarray(bias_b, dtype=np.float32)

    # qT packed: [B, 128, sc*1024 + i] with partitions p = s' within chunk sc
    qT = np.ascontiguousarray(q.transpose(0, 2, 1)).astype(bf16)   # [B, QS, QL]
    qT = np.ascontiguousarray(
        qT.reshape(B, NC_S, 128, QL).transpose(0, 2, 1, 3)).reshape(B, 128, NC_S * QL)
    # padded k, even/odd split: ke[m]=kp[2m], ko[m]=kp[2m+1], m=0..512
    kp = np.zeros((B, KL + 2, KS), dtype=np.float32)
    kp[:, 1:KL + 1, :] = k
    kpb = kp.astype(bf16).astype(np.float32)
    keT = np.ascontiguousarray(kpb[:, 0::2, :].transpose(0, 2, 1))  # [B, KS, 513]
    koT = np.ascontiguousarray(kpb[:, 1::2, :].transpose(0, 2, 1))  # [B, KS, 513]
    kb = np.concatenate([keT.reshape(B, NC_C, 128, KT),
                         koT.reshape(B, NC_C, 128, KT)], axis=3)    # [B, 4, 128, 1026]
    kb = np.ascontiguousarray(kb.transpose(0, 2, 1, 3)).astype(bf16)  # [B, 128, 4, KB]
    # Winograd filter transforms G_u [KS, QS], packed lo/hi halves per c-chunk
    W3 = Wk.reshape(KS, KW, QS)
    g0, g1, g2 = W3[:, 0, :], W3[:, 1, :], W3[:, 2, :]
    G = np.stack([g0, (g0 + g1 + g2) * 0.5, (g0 - g1 + g2) * 0.5, g2], axis=1)
    # [KS, 4u, 2half, 256] -> [KS, half, u, 256]
    G = G.reshape(KS, 4, 2, 256).transpose(0, 2, 1, 3).reshape(KS, 4 * QS)
    G = np.ascontiguousarray(
        G.reshape(NC_C, 128, 4 * QS).transpose(1, 0, 2)).astype(bf16)  # [128, 4, 2048]
    # r[b, j] = sum_{c,t} bk[3c+t] * k_pad[b, j+t, c]  (exact f32, host-added)
    bkr = bk.reshape(KS, KW)
    mm = kp @ bkr                                                  # [B, KL+2, KW]
    r = mm[:, 0:KL, 0] + mm[:, 1:KL + 1, 1] + mm[:, 2:KL + 2, 2]   # [B, KL]
    # bias column: bias[b, i] = q[b] @ Wb[0] + bb + bias_b -> [128, B*NI]
    bias = q @ Wb[0] + (bb[0] + bias_b[0])                         # [B, QL]
    bcc = bias.reshape(B, NI, 128).transpose(2, 0, 1)              # [128, B, NI]

    in_maps = []
    for core in range(NCORES):
        lo = core * B_LOC
        kwg = np.concatenate([kb[lo], G], axis=2)                  # [128, 4, CWK]
        in_maps.append({
            "kwg": np.ascontiguousarray(kwg).reshape(128, NC_C * CWK),
            "kt1": np.ascontiguousarray(kb[lo + 1]).reshape(128, NC_C * KB),
            "qT": np.ascontiguousarray(qT[lo:lo + B_LOC]),
            "bc": np.ascontiguousarray(bcc[:, lo:lo + B_LOC, :]).reshape(128, B_LOC * NI),
        })
    return in_maps, r


def kernel(q, k, Wk, bk, Wb, bb, bias_b):
    nc = _get_nc()
    in_maps, r = _prep_in_maps(q, k, Wk, bk, Wb, bb, bias_b)
    res = run_bass_kernel_spmd(nc, in_maps, list(range(NCORES)))
    dev = np.concatenate(
        [res.results[c]["out"].astype(np.float32) for c in range(NCORES)], axis=0)
    out = np.empty_like(dev)
    out[:, :, 0::2] = dev[:, :, :KL // 2]
    out[:, :, 1::2] = dev[:, :, KL // 2:]
    out += r[:, None, :]
    return out


# revision 10
# speedup vs baseline: 1.0230x; 1.0230x over previous
"""Trainium2 Bass kernel for nn_BatchConv1d (dynamic per-query conv kernels + banded conv).

Reference computation (per batch b):
    G[i, o]   = (q[b] @ Wk.T + bk)[i, o],  o = c*3 + t   (per-query dynamic kernels)
    bias[i]   = (q[b] @ Wb.T + bb)[i, 0]
    scores[i, j] = sum_{c,t} G[i, c*3+t] * k_pad[b, j+t, c]
    out = scores + bias[:, None] + bias_b

Associativity restructure + Winograd F(2,3) on the banded stage:
    stage 1:  N[s, j] = sum_{c,t} Wk[3c+t, s] * k_pad[j+t, c]
              via F(2,3): for output pairs (j=2m, 2m+1) define
                d0=kp[2m], d1=kp[2m+1], d2=kp[2m+2], d3=kp[2m+3]
                D0 = d0-d2, D1 = d1+d2, D2 = d2-d1, D3 = d1-d3   (DVE adds)
                M_u[s, m] = sum_c G_u[c, s] * D_u[m, c]          (4 GEMMs, PE)
                  with G0=g0, G1=(g0+g1+g2)/2, G2=(g0-g1+g2)/2, G3=g2 (host)
                Ne = M0+M1+M2 (even j), No = M1-M2-M3 (odd j)
                  drain: ACT evicts M1,M2 to bf16; DVE combines, with the
                  M0/M3 terms folded into scalar_tensor_tensor PSUM reads
              -> 64 MMs/batch instead of 96 (1.5x fewer stage-1 FLOPs)
    stage 2:  scores = q @ N  (64 MMs/batch); bias[i] rides the eviction
              (even halves on ACT, odd halves on DVE); rank-1 bk-term r[j]
              added on the host after the gather.
    Output is stored j-deinterleaved ([even block | odd block]); the host
    re-interleaves during the gather (free).

Input DMA is consumption-ordered on one ring (delivery is HBM-limited at
~370 GB/s): per c-chunk kb then G-lo (s-chunks 0-1), then the G-hi halves,
then qT/bias/batch-1 k. Batch-0 stage 1 runs (sh0,sh1) jointly contraction-
outermost (each arriving chunk feeds 8 MMs), sh2 contraction-outermost on the
arriving G-hi stream, sh3 u-outermost so its bank drains stagger into stage 2.

Sharding: batch data-parallel, 2 batches per core across 8 NeuronCores.
Compute dtype: bf16 matmul inputs, fp32 PSUM accumulation, bf16 output.
"""
import ml_dtypes
import numpy as np

from concourse import bacc, tile, mybir
from concourse.bass_utils import run_bass_kernel_spmd

BF16 = mybir.dt.bfloat16
F32 = mybir.dt.float32
Identity = mybir.ActivationFunctionType.Identity
ADD = mybir.AluOpType.add
MULT = mybir.AluOpType.mult

B, QL, KL, QS, KS, KW = 16, 1024, 1024, 512, 512, 3
NCORES = 8
B_LOC = B // NCORES      # 2 batches per core
NC_C = KS // 128         # 4 chunks of the c contraction dim
NC_S = QS // 128         # 4 s-chunks
NI = QL // 128           # 8 i-chunks
M = KL // 2              # 512 Winograd output pairs
KT = M + 1               # 513 ke/ko columns (incl. the pad sample)
KB = 2 * KT              # 1026 = ke|ko block per c-chunk
GH = 4 * 256             # 1024 cols per G half (4 u's x 256 s)
CWK = KB + 2 * GH        # 3074 packed kb+G width per c-chunk (batch 0)

_NC_CACHE = {}


def _build():
    nc = bacc.Bacc("TRN2", target_bir_lowering=False, debug=False)
    # host-prepped layouts (bf16 unless noted):
    #   kwg [p=c', c*CWK + 0:513]                    ke of batch 0: kp[2m, 128c+p]
    #       [p=c', c*CWK + 513 + m]                  ko of batch 0: kp[2m+1, .]
    #       [p=c', c*CWK + 1026 + u*256 + s']        G_u[128c+p, s], s in [0,256)
    #       [p=c', c*CWK + 2050 + u*256 + s']        G_u[128c+p, s], s in [256,512)
    #   kt1 [p=c', c*1026 + {ke|ko}]                 batch 1 ke/ko
    #   qT  [b][p=s', sc*1024 + i]                   q transposed, s-major
    #   bc  [p=i', b*8+ih]  f32                      bias col (+bb+bias_b)
    kwg_d = nc.declare_dram_parameter("kwg", [128, NC_C * CWK], BF16, isOutput=False)
    kt1_d = nc.declare_dram_parameter("kt1", [128, NC_C * KB], BF16, isOutput=False)
    qT_d = nc.declare_dram_parameter("qT", [B_LOC, 128, NC_S * QL], BF16, isOutput=False)
    bc_d = nc.declare_dram_parameter("bc", [128, B_LOC * NI], F32, isOutput=False)
    # out cols 0:512 = even j, 512:1024 = odd j (host re-interleaves)
    out_d = nc.declare_dram_parameter("out", [B_LOC, QL, KL], BF16, isOutput=True)

    with tile.TileContext(nc) as tc:
        with (
            tc.tile_pool(name="const", bufs=1) as cpool,
            tc.tile_pool(name="qin", bufs=2) as qpool,
            tc.tile_pool(name="din", bufs=2) as dpool,
            tc.tile_pool(name="cev", bufs=2) as cepool,
            tc.tile_pool(name="tmp", bufs=2) as tpool,
            tc.tile_pool(name="nst", bufs=2) as npool,
            tc.tile_pool(name="outp", bufs=3) as opool,
            tc.tile_pool(name="ps", bufs=2, space="PSUM") as ps,
        ):
            # ---- PE warmup: junk matmuls on a gpsimd-memset tile so the HAM
            #      clock gate flips while the first input chunks are in flight ----
            wz_sb = cpool.tile([128, 640], BF16)
            nc.gpsimd.memset(wz_sb[:], 0.0)
            wps = ps.tile([128, 512], F32, tag="u3", name="wps")
            for _ in range(7):
                nc.tensor.matmul(wps[:], wz_sb[:, 0:128], wz_sb[:, 128:640],
                                 start=True, stop=True)

            # ---- input DMAs: one ring (sync HWDGE), strict FIFO in
            #      consumption order: (kb, G-lo) per c-chunk, G-hi per
            #      c-chunk, then qT0 / bias / batch-1 kb / qT1 ----
            kwg = cpool.tile([128, NC_C * CWK], BF16)
            for c in range(NC_C):
                nc.sync.dma_start(kwg[:, c * CWK:c * CWK + KB],
                                  kwg_d[:, c * CWK:c * CWK + KB])
                nc.sync.dma_start(kwg[:, c * CWK + KB:c * CWK + KB + GH],
                                  kwg_d[:, c * CWK + KB:c * CWK + KB + GH])
            for c in range(NC_C):
                nc.sync.dma_start(kwg[:, c * CWK + KB + GH:(c + 1) * CWK],
                                  kwg_d[:, c * CWK + KB + GH:(c + 1) * CWK])
            qT = {}
            for b in range(B_LOC):
                qT[b] = qpool.tile([128, NC_S * QL], BF16, tag="qTb", name=f"qT{b}")
            nc.sync.dma_start(qT[0][:], qT_d[0, :, :])
            bc_sb = cpool.tile([128, B_LOC * NI], F32)
            nc.sync.dma_start(bc_sb[:], bc_d[:])
            kt1 = cpool.tile([128, NC_C * KB], BF16)
            nc.sync.dma_start(kt1[:], kt1_d[:])
            nc.sync.dma_start(qT[1][:], qT_d[1, :, :])

            def g_ap(u, c, sh):
                off = c * CWK + KB + (sh // 2) * GH + u * 256 + (sh % 2) * 128
                return kwg[:, off:off + 128]

            for b in range(B_LOC):
                def ke_ap(c, lo, hi):
                    base = (c * CWK) if b == 0 else (c * KB)
                    src = kwg if b == 0 else kt1
                    return src[:, base + lo:base + hi]

                def ko_ap(c, lo, hi):
                    base = ((c * CWK) if b == 0 else (c * KB)) + KT
                    src = kwg if b == 0 else kt1
                    return src[:, base + lo:base + hi]

                # ---- D-transform: D_u per c-chunk, 4 ops each, split
                #      DVE/GpSimd so the per-chunk latency halves ----
                D = dpool.tile([128, NC_C * 2048], BF16, tag="D", name=f"D{b}")

                def d_ap(c, u):
                    off = c * 2048 + u * 512
                    return D[:, off:off + 512]

                for c in range(NC_C):
                    nc.vector.tensor_sub(d_ap(c, 0), ke_ap(c, 0, 512), ke_ap(c, 1, 513))
                    nc.vector.tensor_add(d_ap(c, 1), ko_ap(c, 0, 512), ke_ap(c, 1, 513))
                    nc.vector.tensor_sub(d_ap(c, 2), ke_ap(c, 1, 513), ko_ap(c, 0, 512))
                    nc.vector.tensor_sub(d_ap(c, 3), ko_ap(c, 0, 512), ko_ap(c, 1, 513))

                # ---- stage 1: M_u[s,m] accumulated over c into 4 PSUM banks
                #      per s-chunk; drain: ACT evicts M1/M2 to bf16, DVE does
                #      p12=c1+c2, Ne=M0+p12, m12=c1-c2, No=m12-M3 (the M0/M3
                #      reads folded into scalar_tensor_tensor) ----
                Ne, No = {}, {}

                def drain(sh, banks):
                    c1 = cepool.tile([128, 512], BF16, tag="c1", name="c1")
                    c2 = cepool.tile([128, 512], BF16, tag="c2", name="c2")
                    nc.scalar.activation(c1[:], banks[1][:], Identity)
                    nc.scalar.activation(c2[:], banks[2][:], Identity)
                    p12 = tpool.tile([128, 512], BF16, tag="p12")
                    m12 = tpool.tile([128, 512], BF16, tag="m12")
                    ne = npool.tile([128, 512], BF16, tag=f"ne{sh}")
                    no = npool.tile([128, 512], BF16, tag=f"no{sh}")
                    nc.vector.tensor_add(p12[:], c1[:], c2[:])
                    nc.vector.scalar_tensor_tensor(ne[:], banks[0][:], 1.0, p12[:],
                                                   MULT, ADD)
                    nc.vector.tensor_sub(m12[:], c1[:], c2[:])
                    nc.vector.scalar_tensor_tensor(no[:], banks[3][:], -1.0, m12[:],
                                                   MULT, ADD)
                    Ne[sh], No[sh] = ne, no

                def new_banks(sh):
                    return [ps.tile([128, 512], F32, tag=f"u{u}", name=f"n{sh}{u}")
                            for u in range(4)]

                if b == 0:
                    # lead-in: sh0+sh1 jointly, contraction outermost so each
                    # arriving (kb, G-lo) c-chunk feeds 8 MMs immediately
                    nps = {sh: new_banks(sh) for sh in (0, 1)}
                    for c in range(NC_C):
                        for sh in (0, 1):
                            for u in range(4):
                                nc.tensor.matmul(nps[sh][u][:], g_ap(u, c, sh),
                                                 d_ap(c, u),
                                                 start=(c == 0), stop=(c == NC_C - 1))
                    drain(0, nps[0])
                    drain(1, nps[1])
                    # sh2 contraction-outermost, paced by the arriving G-hi
                    banks2 = new_banks(2)
                    for c in range(NC_C):
                        for u in range(4):
                            nc.tensor.matmul(banks2[u][:], g_ap(u, c, 2), d_ap(c, u),
                                             start=(c == 0), stop=(c == NC_C - 1))
                    drain(2, banks2)
                    rest = (3,)
                else:
                    rest = (0, 1, 2, 3)
                for sh in rest:
                    # u-outer so the 4 banks stop staggered and the drain
                    # overlaps this s-chunk's own MMs
                    banks = new_banks(sh)
                    for u in range(4):
                        for c in range(NC_C):
                            nc.tensor.matmul(banks[u][:], g_ap(u, c, sh), d_ap(c, u),
                                             start=(c == 0), stop=(c == NC_C - 1))
                    drain(sh, banks)

                # ---- stage 2: out = q @ N + bias[i]; even halves evicted on
                #      ACT, odd halves on DVE; host re-interleaves ----
                for i in range(NI):
                    last = b == B_LOC - 1 and i == NI - 1
                    osb = opool.tile([128, KL], BF16, tag="osb")
                    col = bc_sb[:, b * NI + i: b * NI + i + 1]
                    if not last:
                        for half in range(2):
                            Nl = Ne if half == 0 else No
                            bank = ps.tile([128, 512], F32,
                                           tag=f"u{2 * (i % 2) + half}")
                            for sc in range(NC_S):
                                nc.tensor.matmul(
                                    bank[:],
                                    qT[b][:, sc * QL + i * 128: sc * QL + (i + 1) * 128],
                                    Nl[sc][:],
                                    start=(sc == 0), stop=(sc == NC_S - 1),
                                )
                            if half == 0:
                                nc.scalar.activation(osb[:, 0:512], bank[:],
                                                     Identity, bias=col)
                            else:
                                nc.vector.tensor_scalar_add(osb[:, 512:1024],
                                                            bank[:], col)
                        nc.sync.dma_start(out_d[b, i * 128:(i + 1) * 128, :], osb[:])
                    else:
                        # final tile: four N=256 quarter-chains so the
                        # epilogue+store pipeline drains right behind the MMs;
                        # even quarters on ACT+sync ring, odd on DVE+scalar ring
                        for half in range(2):
                            Nl = Ne if half == 0 else No
                            for qh in range(2):
                                bank = ps.tile([128, 256], F32,
                                               tag=f"u{2 * half + qh}",
                                               name=f"fin{half}{qh}")
                                for sc in range(NC_S):
                                    nc.tensor.matmul(
                                        bank[:],
                                        qT[b][:, sc * QL + i * 128:
                                               sc * QL + (i + 1) * 128],
                                        Nl[sc][:, qh * 256:(qh + 1) * 256],
                                        start=(sc == 0), stop=(sc == NC_S - 1),
                                    )
                                lo = half * 512 + qh * 256
                                if half == 0:
                                    nc.scalar.activation(osb[:, lo:lo + 256],
                                                         bank[:], Identity, bias=col)
                                else:
                                    nc.vector.tensor_scalar_add(osb[:, lo:lo + 256],
                                                                bank[:], col)
                                # one ring per quarter so no descriptor-gen
                                # queues behind another at the tail
                                ring = (nc.sync, nc.sync, nc.scalar,
                                        nc.gpsimd)[2 * half + qh]
                                ring.dma_start(
                                    out_d[b, i * 128:(i + 1) * 128, lo:lo + 256],
                                    osb[:, lo:lo + 256],
                                )
    nc.finalize()
    return nc


def _get_nc():
    if "nc" not in _NC_CACHE:
        _NC_CACHE["nc"] = _build()
    return _NC_CACHE["nc"]


def _prep_in_maps(q, k, Wk, bk, Wb, bb, bias_b):
    """Returns (in_maps, r) where r[B, KL] must be added to the reordered output."""
    bf16 = ml_dtypes.bfloat16
    q = np.asarray(q, dtype=np.float32)
    k = np.asarray(k, dtype=np.float32)
    Wk = np.asarray(Wk, dtype=np.float32)
    bk = np.asarray(bk, dtype=np.float32)
    Wb = np.asarray(Wb, dtype=np.float32)
    bb = np.asarray(bb, dtype=np.float32)
    bias_b = np.asarray(bias_b, dtype=np.float32)

    # qT packed: [B, 128, sc*1024 + i] with partitions p = s' within chunk sc
    qT = np.ascontiguousarray(q.transpose(0, 2, 1)).astype(bf16)   # [B, QS, QL]
    qT = np.ascontiguousarray(
        qT.reshape(B, NC_S, 128, QL).transpose(0, 2, 1, 3)).reshape(B, 128, NC_S * QL)
    # padded k, even/odd split: ke[m]=kp[2m], ko[m]=kp[2m+1], m=0..512
    kp = np.zeros((B, KL + 2, KS), dtype=np.float32)
    kp[:, 1:KL + 1, :] = k
    kpb = kp.astype(bf16).astype(np.float32)
    keT = np.ascontiguousarray(kpb[:, 0::2, :].transpose(0, 2, 1))  # [B, KS, 513]
    koT = np.ascontiguousarray(kpb[:, 1::2, :].transpose(0, 2, 1))  # [B, KS, 513]
    kb = np.concatenate([keT.reshape(B, NC_C, 128, KT),
                         koT.reshape(B, NC_C, 128, KT)], axis=3)    # [B, 4, 128, 1026]
    kb = np.ascontiguousarray(kb.transpose(0, 2, 1, 3)).astype(bf16)  # [B, 128, 4, KB]
    # Winograd filter transforms G_u [KS, QS], packed lo/hi halves per c-chunk
    W3 = Wk.reshape(KS, KW, QS)
    g0, g1, g2 = W3[:, 0, :], W3[:, 1, :], W3[:, 2, :]
    G = np.stack([g0, (g0 + g1 + g2) * 0.5, (g0 - g1 + g2) * 0.5, g2], axis=1)
    # [KS, 4u, 2half, 256] -> [KS, half, u, 256]
    G = G.reshape(KS, 4, 2, 256).transpose(0, 2, 1, 3).reshape(KS, 4 * QS)
    G = np.ascontiguousarray(
        G.reshape(NC_C, 128, 4 * QS).transpose(1, 0, 2)).astype(bf16)  # [128, 4, 2048]
    # r[b, j] = sum_{c,t} bk[3c+t] * k_pad[b, j+t, c]  (exact f32, host-added)
    bkr = bk.reshape(KS, KW)
    mm = kp @ bkr                                                  # [B, KL+2, KW]
    r = mm[:, 0:KL, 0] + mm[:, 1:KL + 1, 1] + mm[:, 2:KL + 2, 2]   # [B, KL]
    # bias column: bias[b, i] = q[b] @ Wb[0] + bb + bias_b -> [128, B*NI]
    bias = q @ Wb[0] + (bb[0] + bias_b[0])                         # [B, QL]
    bcc = bias.reshape(B, NI, 128).transpose(2, 0, 1)              # [128, B, NI]

    in_maps = []
    for core in range(NCORES):
        lo = core * B_LOC
        kwg = np.concatenate([kb[lo], G], axis=2)                  # [128, 4, CWK]
        in_maps.append({
            "kwg": np.ascontiguousarray(kwg).reshape(128, NC_C * CWK),
            "kt1": np.ascontiguousarray(kb[lo + 1]).reshape(128, NC_C * KB),
            "qT": np.ascontiguousarray(qT[lo:lo + B_LOC]),
            "bc": np.ascontiguousarray(bcc[:, lo:lo + B_LOC, :]).reshape(128, B_LOC * NI),
        })
    return in_maps, r


def kernel(q, k, Wk, bk, Wb, bb, bias_b):
    nc = _get_nc()
    in_maps, r = _prep_in_maps(q, k, Wk, bk, Wb, bb, bias_b)
    res = run_bass_kernel_spmd(nc, in_maps, list(range(NCORES)))
    dev = np.concatenate(
        [res.results[c]["out"].astype(np.float32) for c in range(NCORES)], axis=0)
    out = np.empty_like(dev)
    out[:, :, 0::2] = dev[:, :, :KL // 2]
    out[:, :, 1::2] = dev[:, :, KL // 2:]
    out += r[:, None, :]
    return out
